# revision 23
# baseline (speedup 1.0000x reference)
"""BarrierNet Trainium2 kernel: MLP + batched 2-var QP (active-set enumeration).

Self-contained: shards B=262144 samples across 8 NeuronCores (data parallel),
runs a Bass/Tile kernel per core, gathers the full output.

v2 layout per core (SHARD = 32768 samples):
  - MLP: 64 tiles of 512 samples, feature-major fp32r matmuls, software
    pipelined so the PE never waits (L1(t+1) | L2(t) | L3(t-1) interleave).
    xT comes pre-transposed from the host and is DMA'd straight into f32r.
  - All x-only QP geometry (G, q, S, barrier, det, reciprocal dens, wide
    operand arrangements) is "dripped" between MLP tiles so DVE/ACT/Pool
    fill their idle time under the PE-bound phase.
  - The z-dependent QP tail (p, sigmoids -> hq -> 11-candidate enumeration)
    compares candidates on E = |u - u0|^2 (obj(u) = -0.5|p|^2 + 0.5 E) and
    is engine-balanced with a measured-cost greedy.
"""
import numpy as np

import concourse.bass as bass
import concourse.bacc as bacc
import concourse.tile as tile
from concourse import mybir
from concourse.bass_utils import run_bass_kernel_spmd

F32 = mybir.dt.float32
F32R = mybir.dt.float32r
U8 = mybir.dt.uint8
Alu = mybir.AluOpType
Act = mybir.ActivationFunctionType

NCORES = 8
B = 262144
SHARD = B // NCORES            # 32768
PC = SHARD // 128              # 256  (plane free dim)
TILE_N = 512
NT = SHARD // TILE_N           # 64
NF, H1, H2 = 8, 256, 128

STATIC_OBS = np.array([[8.0, -8.0, 1.0], [-9.0, 7.0, 1.0], [10.0, 10.0, 1.5]],
                      np.float32)
AGENT_R, SAFETY = np.float32(0.5), np.float32(0.1)
TOL = 1e-6
PAIRS = [(0, 1), (0, 2), (0, 3), (1, 2), (1, 3), (2, 3)]

_NC_CACHE = {}


def _f(x):
    return float(np.float32(x))


def _build_nc(zero_bias=False):
    nc = bacc.Bacc("TRN2", target_bir_lowering=False, debug=False,
                   num_devices=NCORES)

    x_d = nc.dram_tensor("x", [SHARD, NF], F32, kind="ExternalInput")
    xt_d = nc.dram_tensor("xt", [NF, SHARD], F32R, kind="ExternalInput")
    w1t_d = nc.dram_tensor("w1t", [NF, H1], F32R, kind="ExternalInput")
    wcat_d = nc.dram_tensor("wcat", [128, 512], F32R, kind="ExternalInput")
    w3t_d = nc.dram_tensor("w3t", [128, 8], F32R, kind="ExternalInput")
    consts_d = nc.dram_tensor("consts", [128, 16], F32, kind="ExternalInput")
    out_d = nc.dram_tensor("out", [SHARD, 2], F32, kind="ExternalOutput")

    W4, W6 = 4 * PC, 6 * PC

    with tile.TileContext(nc) as tc:
        with tc.tile_pool(name="wpool", bufs=1) as wp, \
             tc.tile_pool(name="mlp", bufs=2) as mp, \
             tc.tile_pool(name="planes", bufs=1) as pp, \
             tc.tile_pool(name="tmps", bufs=2) as tp, \
             tc.tile_pool(name="psA", bufs=2, space="PSUM") as psA, \
             tc.tile_pool(name="psB", bufs=1, space="PSUM") as psB, \
             tc.tile_pool(name="psC", bufs=2, space="PSUM") as psC, \
             tc.tile_pool(name="dram", bufs=1, space="DRAM") as dp:

            # ---------------- engine-balance helpers ----------------
            eng_load = {"v": 0.0, "a": 0.0, "g": 0.0}
            POOL_TT_OK = (Alu.add, Alu.subtract, Alu.mult)

            def _fd(ap):
                return ap.free_size()

            def _pick(costs):
                e = min(costs, key=lambda k: eng_load[k] + costs[k])
                eng_load[e] += costs[e]
                return e

            def _take(e, cost):
                eng_load[e] += cost

            def e_tt(out, a, b, op, eng=None):
                """tensor-tensor; min/cmp only on DVE; bcast only on DVE."""
                fd = _fd(out)
                if eng is None:
                    if op in POOL_TT_OK:
                        eng = _pick({"v": 60 + fd * 1.042,
                                     "g": 100 + fd * 2.083})
                    else:
                        eng = "v"
                        _take("v", 60 + fd * 1.042)
                else:
                    _take(eng, 60 + fd * 1.042 if eng == "v"
                          else 100 + fd * 2.083)
                (nc.vector if eng == "v" else nc.gpsimd).tensor_tensor(
                    out, a, b, op)
                return out

            def e_ts(out, a, s1, s2, op0, op1=None, eng=None):
                fd = _fd(out)
                s1 = s1 if isinstance(s1, bass.AP) else _f(s1)
                if s2 is not None:
                    s2 = s2 if isinstance(s2, bass.AP) else _f(s2)
                if eng is None:
                    eng = _pick({"v": 60 + fd * 0.521, "g": 100 + fd * 1.48})
                else:
                    _take(eng, 60 + fd * 0.521 if eng == "v"
                          else 100 + fd * 1.48)
                e = nc.vector if eng == "v" else nc.gpsimd
                if s2 is None:
                    e.tensor_scalar(out, a, s1, None, op0)
                else:
                    e.tensor_scalar(out, a, s1, s2, op0, op1)
                return out

            def e_stt(out, a, s, b, op0, op1):
                _take("v", 60 + _fd(out) * 1.042)
                nc.vector.scalar_tensor_tensor(out, a, _f(s), b, op0, op1)
                return out

            def e_un(out, a, func, bias=0.0, scale=1.0):
                _take("a", 255 + _fd(out) * 0.833)
                nc.scalar.activation(out, a, func, bias=bias, scale=scale)
                return out

            def e_sq(out, a, scale=None):
                fd = _fd(out)
                if scale is None:
                    eng = _pick({"v": 60 + fd * 1.042,
                                 "g": 100 + fd * 2.083,
                                 "a": 255 + fd * 0.833})
                else:
                    eng = "a"
                    _take("a", 255 + fd * 0.833)
                if eng == "a":
                    nc.scalar.activation(out, a, Act.Square,
                                         scale=1.0 if scale is None else scale)
                else:
                    (nc.vector if eng == "v" else nc.gpsimd).tensor_tensor(
                        out, a, a, Alu.mult)
                return out

            def e_copy(out, a, eng=None):
                fd = _fd(out)
                if eng is None:
                    eng = _pick({"v": 60 + fd * 0.521,
                                 "g": 100 + fd * 1.48,
                                 "a": 255 + fd * 0.833})
                else:
                    _take(eng, 100 + fd * 1.48)
                if eng == "a":
                    nc.scalar.activation(out, a, Act.Copy)
                else:
                    (nc.vector if eng == "v" else nc.gpsimd).tensor_copy(out, a)
                return out

            def e_recip(out, a):
                _take("v", 60 + _fd(out) * 1.042)
                nc.vector.reciprocal_approx_fast(out, a)
                return out

            # ---------------- tiles: constants / weights ----------------
            cs = wp.tile([128, 16], F32, tag="consts", name="consts")
            nc.sync.dma_start(out=cs[:], in_=consts_d.ap())
            w1t_r = wp.tile([NF, H1], F32R, tag="w1tr", name="w1tr")
            wcat_r = wp.tile([128, 512], F32R, tag="wcatr", name="wcatr")
            w3t_r = wp.tile([128, 8], F32R, tag="w3tr", name="w3tr")
            nc.sync.dma_start(out=w1t_r[:], in_=w1t_d.ap())
            nc.sync.dma_start(out=wcat_r[:], in_=wcat_d.ap())
            nc.sync.dma_start(out=w3t_r[:], in_=w3t_d.ap())

            # x planes (f-interleaved) for geometry
            xbig = wp.tile([128, PC * NF], F32, tag="xbig", name="xbig")
            nc.sync.dma_start(
                out=xbig[:], in_=x_d.ap().rearrange("(p c) f -> p (c f)", p=128))
            xb3 = xbig[:].rearrange("p (c f) -> p c f", f=NF)

            def xf(i):
                return xb3[:, :, i]

            z3_dram = dp.tile([4, SHARD], F32, tag="z3d", name="z3d")

            # ---------------- persistent QP tiles ----------------
            def w4t(tag):
                return pp.tile([128, W4], F32, tag=tag, name=tag)

            def w6t(tag):
                return pp.tile([128, W6], F32, tag=tag, name=tag)

            def plane(tag):
                return pp.tile([128, PC], F32, tag=tag, name=tag)

            def tmp():
                tmp._i += 1
                t = f"tmp{tmp._i % 6}"
                return tp.tile([128, PC], F32, tag=t, name=t, bufs=2)

            tmp._i = 0

            def slab(w, i):
                return w[:, i * PC:(i + 1) * PC]

            def bc(plane_ap, n):
                return plane_ap.rearrange(
                    "p (o c) -> p o c", o=1).to_broadcast((128, n, PC))

            def w3v(w, n=4):
                return w[:].rearrange("p (o c) -> p o c", o=n)

            # x-only wide4: A (bd), BAR16, G1W, G2W, QW, RDENW
            AW, BARW = w4t("AW"), w4t("BARW")
            G1W, G2W = w4t("G1W"), w4t("G2W")
            QW, RDENW = w4t("QW"), w4t("RDENW")
            # x-only wide6
            RDS6, OKF6 = w6t("RDS6"), w6t("OKF6")
            G1M1, G2M1 = w6t("G1M1"), w6t("G2M1")
            G1M2, G2M2 = w6t("G1M2"), w6t("G2M2")
            OKI6 = pp.tile([128, W6], U8, tag="OKI6", name="OKI6")

            # ---------------- x-only geometry (deferred drip) ----------
            XOPS = []

            def xop(f):
                XOPS.append(f)

            mu = [0.0, 0.0, 0.0, 1.0, 6.0, 6.0]
            sg = [1.0, 1.0, 0.5, 0.3, 1.0, 1.0]
            rtot = np.concatenate(
                [AGENT_R + STATIC_OBS[:, 2] + SAFETY,
                 np.array([2 * AGENT_R + SAFETY], np.float32)]).astype(np.float32)
            r2 = (rtot * rtot).astype(np.float32)

            DXW, DYW = w4t("DXW"), w4t("DYW")
            st, ct, v = plane("st"), plane("ct"), plane("v")
            vst, vct, h0 = plane("vst"), plane("vct"), plane("h0")
            oxo, oyo, sh, sh2 = tmp(), tmp(), tmp(), tmp()

            for k in range(3):
                xop(lambda k=k: e_ts(slab(DXW, k), xf(0), sg[0],
                                     mu[0] - STATIC_OBS[k, 0], Alu.mult,
                                     Alu.add, eng="g"))
                xop(lambda k=k: e_ts(slab(DYW, k), xf(1), sg[1],
                                     mu[1] - STATIC_OBS[k, 1], Alu.mult,
                                     Alu.add, eng="g"))
            xop(lambda: e_ts(oxo[:], xf(4), sg[4], mu[4] - mu[0], Alu.mult,
                             Alu.add, eng="g"))
            xop(lambda: e_ts(oyo[:], xf(5), sg[5], mu[5] - mu[1], Alu.mult,
                             Alu.add, eng="g"))
            # dx_opp = x0*sg0 - oxo  (no Pool STT: two steps via slab reuse)
            xop(lambda: e_ts(slab(DXW, 3), xf(0), sg[0], None, Alu.mult,
                             eng="g"))
            xop(lambda: e_tt(slab(DXW, 3), slab(DXW, 3), oxo[:], Alu.subtract,
                             eng="g"))
            xop(lambda: e_ts(slab(DYW, 3), xf(1), sg[1], None, Alu.mult,
                             eng="g"))
            xop(lambda: e_tt(slab(DYW, 3), slab(DYW, 3), oyo[:], Alu.subtract,
                             eng="g"))

            xop(lambda: e_un(st[:], xf(2), Act.Sin, bias=0.0, scale=sg[2]))
            xop(lambda: e_un(sh[:], xf(2), Act.Sin, bias=0.0,
                             scale=sg[2] * 0.5))
            xop(lambda: e_tt(sh2[:], sh[:], sh[:], Alu.mult, eng="g"))
            xop(lambda: e_ts(ct[:], sh2[:], -2.0, 1.0, Alu.mult, Alu.add,
                             eng="g"))
            xop(lambda: e_ts(v[:], xf(3), sg[3], float(mu[3]), Alu.mult,
                             Alu.add, eng="g"))
            xop(lambda: e_tt(vst[:], v[:], st[:], Alu.mult, eng="g"))
            xop(lambda: e_tt(vct[:], v[:], ct[:], Alu.mult, eng="g"))
            # h0 = 2 v^2
            xop(lambda: e_tt(h0[:], v[:], v[:], Alu.mult, eng="g"))
            xop(lambda: e_ts(h0[:], h0[:], 2.0, None, Alu.mult, eng="g"))

            # per-slab products: A = dx*vct + dy*vst ; G1 = dy*vct - dx*vst
            #                    G2 = dx*ct + dy*st
            for k in range(4):
                xop(lambda k=k: e_tt(slab(AW, k), slab(DXW, k), vct[:],
                                     Alu.mult, eng="g"))
                xop(lambda k=k: e_tt(slab(G1W, k), slab(DYW, k), vct[:],
                                     Alu.mult, eng="g"))
                xop(lambda k=k: e_tt(slab(G2W, k), slab(DXW, k), ct[:],
                                     Alu.mult, eng="g"))
            T1, T2 = tmp(), tmp()
            for k in range(4):
                xop(lambda k=k: e_tt(T1[:], slab(DYW, k), vst[:], Alu.mult,
                                     eng="g"))
                xop(lambda k=k: e_tt(slab(AW, k), slab(AW, k), T1[:], Alu.add,
                                     eng="g"))
                xop(lambda k=k: e_tt(T2[:], slab(DXW, k), vst[:], Alu.mult,
                                     eng="g"))
                xop(lambda k=k: e_tt(slab(G1W, k), slab(G1W, k), T2[:],
                                     Alu.subtract, eng="g"))
                xop(lambda k=k: e_tt(T1[:], slab(DYW, k), st[:], Alu.mult,
                                     eng="g"))
                xop(lambda k=k: e_tt(slab(G2W, k), slab(G2W, k), T1[:],
                                     Alu.add, eng="g"))
            # BAR16 = 16*(dx^2 + dy^2 - r2)
            for k in range(4):
                xop(lambda k=k: e_tt(slab(BARW, k), slab(DXW, k), slab(DXW, k),
                                     Alu.mult, eng="g"))
                xop(lambda k=k: e_tt(T2[:], slab(DYW, k), slab(DYW, k),
                                     Alu.mult, eng="g"))
                xop(lambda k=k: e_tt(slab(BARW, k), slab(BARW, k), T2[:],
                                     Alu.add, eng="g"))
                xop(lambda k=k: e_ts(slab(BARW, k), slab(BARW, k), 16.0,
                                     _f(-16.0 * r2[k]), Alu.mult, Alu.add,
                                     eng="g"))
            # QW = g1^2 + g2^2 ; RDEN = 1/(2q + eps)
            for k in range(4):
                xop(lambda k=k: e_tt(slab(QW, k), slab(G1W, k), slab(G1W, k),
                                     Alu.mult, eng="g"))
                xop(lambda k=k: e_tt(T2[:], slab(G2W, k), slab(G2W, k),
                                     Alu.mult, eng="g"))
                xop(lambda k=k: e_tt(slab(QW, k), slab(QW, k), T2[:], Alu.add,
                                     eng="g"))
            xop(lambda: e_ts(RDENW[:], QW[:], 2.0, 5e-10, Alu.mult, Alu.add,
                             eng="g"))
            xop(lambda: e_recip(RDENW[:], RDENW[:]))

            # S planes (j<k pairs)
            Spl = {}
            for (i, j) in PAIRS:
                s_ij = plane(f"S{i}{j}")
                Spl[(i, j)] = Spl[(j, i)] = s_ij
                xop(lambda i=i, j=j, s=s_ij: e_tt(s[:], slab(G1W, i),
                                                  slab(G1W, j), Alu.mult,
                                                  eng="g"))
                xop(lambda i=i, j=j, s=s_ij: e_tt(T1[:], slab(G2W, i),
                                                  slab(G2W, j), Alu.mult,
                                                  eng="g"))
                xop(lambda s=s_ij: e_tt(s[:], s[:], T1[:], Alu.add, eng="g"))

            # u2 det stage: det/4 = g1i*g2j - g2i*g1j ; RDS6 = 1/det_safe
            DET6 = w6t("DET6")
            for pi, (i, j) in enumerate(PAIRS):
                xop(lambda pi=pi, i=i, j=j: e_tt(slab(DET6, pi), slab(G1W, i),
                                                 slab(G2W, j), Alu.mult,
                                                 eng="g"))
                xop(lambda pi=pi, i=i, j=j: e_tt(T2[:], slab(G2W, i),
                                                 slab(G1W, j), Alu.mult,
                                                 eng="g"))
                xop(lambda pi=pi: e_tt(slab(DET6, pi), slab(DET6, pi), T2[:],
                                       Alu.subtract, eng="g"))
            xop(lambda: e_un(OKF6[:], DET6[:], Act.Abs))
            xop(lambda: e_ts(OKI6[:], OKF6[:], 2.5e-10, None, Alu.is_gt,
                             eng="g"))
            xop(lambda: e_ts(OKF6[:], OKF6[:], 2.5e-10, None, Alu.is_gt,
                             eng="g"))
            xop(lambda: e_ts(RDS6[:], OKF6[:], 0.0, 0.25, Alu.mult, Alu.add,
                             eng="g"))
            xop(lambda: (_take("v", 60 + W6 * 1.042),
                         nc.vector.copy_predicated(RDS6[:], OKI6[:],
                                                   DET6[:]))[1])
            xop(lambda: e_recip(RDS6[:], RDS6[:]))

            # feas-u2 G arrangements: for pair (i,j), m1/m2 = constraints
            # not in the pair
            MSETS = [[m for m in range(4) if m not in pr] for pr in PAIRS]
            for pi in range(6):
                m1, m2 = MSETS[pi]
                xop(lambda pi=pi, m=m1: e_copy(slab(G1M1, pi), slab(G1W, m)))
                xop(lambda pi=pi, m=m1: e_copy(slab(G2M1, pi), slab(G2W, m)))
                xop(lambda pi=pi, m=m2: e_copy(slab(G1M2, pi), slab(G1W, m)))
                xop(lambda pi=pi, m=m2: e_copy(slab(G2M2, pi), slab(G2W, m)))

            # ---------------- MLP pipelined loop ----------------
            ACT_SPLIT = 576          # relu cols on ACT; rest on DVE
            xt_tiles = [None] * NT
            h1ps = [None] * NT
            h2ps = [None] * NT
            h1r = [None] * NT
            h2r = [None] * NT
            z3ps = [None] * (NT // 2)

            emitted = [0]

            def drip(i):
                quota = (len(XOPS) * (i + 1)) // (NT + 2)
                while emitted[0] < quota:
                    XOPS[emitted[0]]()
                    emitted[0] += 1

            for i in range(NT + 2):
                if i < NT:
                    xt = mp.tile([NF, TILE_N], F32R, tag="xTr", name="xTr",
                                 bufs=2)
                    nc.sync.dma_start(
                        out=xt[:],
                        in_=xt_d.ap()[:, i * TILE_N:(i + 1) * TILE_N])
                    xt_tiles[i] = xt
                    h1_ps = psA.tile([128, 2 * TILE_N], F32, tag="h1ps",
                                     name="h1ps")
                    for mc in range(2):
                        nc.tensor.matmul(
                            h1_ps[:, mc * TILE_N:(mc + 1) * TILE_N],
                            w1t_r[:, mc * 128:(mc + 1) * 128],
                            xt[:], start=True, stop=True)
                    h1ps[i] = h1_ps
                if 1 <= i:
                    t = i - 1
                    if t < NT:
                        # h1 epilogue split ACT/DVE
                        h1r_t = mp.tile([128, 2 * TILE_N], F32R, tag="h1r",
                                        name="h1r")
                        hp = h1ps[t]
                        if zero_bias:
                            nc.scalar.activation(h1r_t[:, :ACT_SPLIT],
                                                 hp[:, :ACT_SPLIT], Act.Relu)
                            nc.vector.tensor_scalar(h1r_t[:, ACT_SPLIT:],
                                                    hp[:, ACT_SPLIT:],
                                                    0.0, None, Alu.max)
                        else:
                            nc.scalar.activation(h1r_t[:, 0:TILE_N],
                                                 hp[:, 0:TILE_N], Act.Relu,
                                                 bias=cs[:, 0:1], scale=1.0)
                            nc.vector.tensor_scalar(h1r_t[:, TILE_N:],
                                                    hp[:, TILE_N:],
                                                    cs[:, 1:2], 0.0,
                                                    Alu.add, Alu.max)
                        h1r[t] = h1r_t
                        h1ps[t] = None
                        # L2
                        h2_ps = psB.tile([128, 2 * TILE_N], F32, tag="h2ps",
                                         name="h2ps")
                        for mo in range(2):
                            for kc in range(2):
                                nc.tensor.matmul(
                                    h2_ps[:, mo * TILE_N:(mo + 1) * TILE_N],
                                    wcat_r[:, kc * 256 + mo * 128:
                                           kc * 256 + (mo + 1) * 128],
                                    h1r_t[:, kc * TILE_N:(kc + 1) * TILE_N],
                                    start=(kc == 0), stop=(kc == 1))
                        h2ps[t] = h2_ps
                if 2 <= i:
                    t = i - 2
                    if t < NT:
                        # h2 epilogue split ACT/DVE
                        h2r_t = mp.tile([128, 2 * TILE_N], F32R, tag="h2r",
                                        name="h2r")
                        hp = h2ps[t]
                        if zero_bias:
                            nc.scalar.activation(h2r_t[:, :ACT_SPLIT],
                                                 hp[:, :ACT_SPLIT], Act.Relu)
                            nc.vector.tensor_scalar(h2r_t[:, ACT_SPLIT:],
                                                    hp[:, ACT_SPLIT:],
                                                    0.0, None, Alu.max)
                        else:
                            nc.scalar.activation(h2r_t[:, 0:TILE_N],
                                                 hp[:, 0:TILE_N], Act.Relu,
                                                 bias=cs[:, 2:3], scale=1.0)
                            nc.vector.tensor_scalar(h2r_t[:, TILE_N:],
                                                    hp[:, TILE_N:],
                                                    cs[:, 3:4], 0.0,
                                                    Alu.add, Alu.max)
                        h2r[t] = h2r_t
                        h2ps[t] = None
                        h1r[t] = None
                        # L3
                        zp = psC.tile([4, TILE_N], F32, tag="z3ps",
                                      name="z3ps")
                        for kc in range(2):
                            nc.tensor.matmul(
                                zp[:],
                                w3t_r[:, kc * 4:(kc + 1) * 4],
                                h2r_t[:, kc * TILE_N:(kc + 1) * TILE_N],
                                start=(kc == 0), stop=(kc == 1))
                        z3_sb = mp.tile([4, TILE_N], F32, tag="z3sb",
                                        name="z3sb")
                        if t % 2 == 0:
                            nc.vector.tensor_copy(z3_sb[:], zp[:])
                        else:
                            nc.scalar.activation(z3_sb[:], zp[:], Act.Copy)
                        nc.sync.dma_start(
                            out=z3_dram[:, t * TILE_N:(t + 1) * TILE_N],
                            in_=z3_sb[:])
                drip(i)

            # ============ z-dependent QP tail ============
            # reset balancer (all engines free now)
            for k in eng_load:
                eng_load[k] = 0.0

            zpl = []
            for iz in range(4):
                z = tmp()
                nc.sync.dma_start(
                    out=z[:],
                    in_=z3_dram[iz, :].rearrange("(p c) -> p c", p=128))
                zpl.append(z)
            zr1, zr2, zs1, zs2 = (z[:] for z in zpl)

            p1t, p2t = plane("p1"), plane("p2")
            e_un(p1t[:], zr1, Act.Identity, bias=cs[:, 13:14])
            e_un(p2t[:], zr2, Act.Identity, bias=cs[:, 14:15])
            p1, p2 = p1t[:], p2t[:]
            sg1, sg2p = plane("sg1"), plane("sg2")
            e_un(sg1[:], zs1, Act.Sigmoid, bias=cs[:, 15:16])
            e_un(sg2p[:], zs2, Act.Sigmoid, bias=cs[:, 4:5])
            sab8, smm = plane("sab8"), plane("smm")
            e_tt(sab8[:], sg1[:], sg2p[:], Alu.add)
            e_ts(sab8[:], sab8[:], 8.0, None, Alu.mult)
            e_tt(smm[:], sg1[:], sg2p[:], Alu.mult)

            # wide z-dep buffers
            HQW, HTW = w4t("HQW"), w4t("HTW")
            M0W, LAMW = w4t("M0W"), w4t("LAMW")
            U1XW, U1YW = DXW, DYW          # reuse (geometry dead)
            C4, D4 = w4t("C4"), w4t("D4")

            # HQ = h0 + sab8*A + smm*BAR16
            e_tt(w3v(HQW), bc(sab8[:], 4), w3v(AW), Alu.mult)
            e_tt(w3v(C4), bc(smm[:], 4), w3v(BARW), Alu.mult)
            e_tt(HQW[:], HQW[:], C4[:], Alu.add)
            e_tt(w3v(HQW), w3v(HQW), bc(h0[:], 4), Alu.add)
            e_ts(HTW[:], HQW[:], 0.5, _f(TOL * 0.5), Alu.mult, Alu.add)

            # t_k = g1*p1 + g2*p2 ; M0 = HT - t ; nu = 2t - hq ; lam = nu*rden
            e_tt(w3v(C4), w3v(G1W), bc(p1, 4), Alu.mult)
            e_tt(w3v(D4), w3v(G2W), bc(p2, 4), Alu.mult)
            e_tt(C4[:], C4[:], D4[:], Alu.add)                 # t_k
            e_tt(M0W[:], HTW[:], C4[:], Alu.subtract)
            e_stt(C4[:], C4[:], 2.0, HQW[:], Alu.mult, Alu.subtract)  # nu
            e_tt(LAMW[:], C4[:], RDENW[:], Alu.mult)
            MKW = RDENW                    # reuse (rden dead after lam)

            # candidate 0
            fmin0, flag0 = tmp(), tmp()
            nc.vector.tensor_reduce(
                fmin0[:], M0W[:].rearrange("p (k c) -> p c k", k=4),
                mybir.AxisListType.X, Alu.min)
            _take("v", 60 + W4 * 1.042)
            bo, bx, by = plane("best_obj"), plane("best_ux"), plane("best_uy")
            e_ts(flag0[:], fmin0[:], 0.0, None, Alu.is_ge)
            e_ts(bo[:], flag0[:], -1e30, 1e30, Alu.mult, Alu.add)
            e_ts(bx[:], p1, -1.0, None, Alu.mult)
            e_ts(by[:], p2, -1.0, None, Alu.mult)

            def fold_candidate(objm, ux, uy):
                bt = tp.tile([128, PC], U8, tag="bt", name="bt", bufs=2)
                e_tt(bt[:], objm, bo[:], Alu.is_lt, eng="v")
                _take("v", 2 * (60 + PC * 1.042))
                nc.vector.copy_predicated(bx[:], bt[:], ux)
                nc.vector.copy_predicated(by[:], bt[:], uy)
                e_tt(bo[:], objm, bo[:], Alu.min, eng="v")

            # u1 candidates
            e_tt(U1XW[:], LAMW[:], G1W[:], Alu.mult)
            e_tt(w3v(U1XW), w3v(U1XW), bc(p1, 4), Alu.subtract)
            e_tt(U1YW[:], LAMW[:], G2W[:], Alu.mult)
            e_tt(w3v(U1YW), w3v(U1YW), bc(p2, 4), Alu.subtract)
            e_ts(MKW[:], LAMW[:], -_f(2 * TOL), None, Alu.is_ge)  # dual flag

            # u1 feasibility via S: margin_j = M0_j + lam_k * S_jk
            for k in range(4):
                fm = None
                for j in range(4):
                    if j == k:
                        continue
                    e1 = tmp()
                    e_tt(e1[:], slab(LAMW, k), Spl[(j, k)][:], Alu.mult)
                    mg = tmp()
                    e_tt(mg[:], e1[:], slab(M0W, j), Alu.add)
                    if fm is None:
                        fm = mg
                    else:
                        e_tt(fm[:], fm[:], mg[:], Alu.min, eng="v")
                ff = tmp()
                e_ts(ff[:], fm[:], 0.0, None, Alu.is_ge)
                e_tt(slab(MKW, k), slab(MKW, k), ff[:], Alu.mult)

            # E = lam^2 * q, masked
            e_sq(C4[:], LAMW[:])
            e_tt(C4[:], C4[:], QW[:], Alu.mult)
            e_ts(D4[:], MKW[:], -1e30, 1e30, Alu.mult, Alu.add)
            e_tt(C4[:], C4[:], D4[:], Alu.add)
            for k in range(4):
                fold_candidate(slab(C4, k), slab(U1XW, k), slab(U1YW, k))

            # ---------------- u2 candidates (6 pairs) ----------------
            U2XW, U2YW = w6t("U2XW"), w6t("U2YW")
            W0, W1 = w6t("W0"), w6t("W1")
            LH, LI2 = w6t("LH"), w6t("LI2")
            EM6 = DET6                                  # reuse (det dead)

            # e = hq_i g2_j - hq_j g2_i ; e2 = g1_j hq_i - g1_i hq_j
            for pi, (i, j) in enumerate(PAIRS):
                t1 = tmp()
                e_tt(t1[:], slab(HQW, i), slab(G2W, j), Alu.mult)
                t2 = tmp()
                e_tt(t2[:], slab(HQW, j), slab(G2W, i), Alu.mult)
                e_tt(slab(U2XW, pi), t1[:], t2[:], Alu.subtract)
                t3 = tmp()
                e_tt(t3[:], slab(G1W, j), slab(HQW, i), Alu.mult)
                t4 = tmp()
                e_tt(t4[:], slab(G1W, i), slab(HQW, j), Alu.mult)
                e_tt(slab(U2YW, pi), t3[:], t4[:], Alu.subtract)
            e_stt(U2XW[:], U2XW[:], -0.5, RDS6[:], Alu.mult, Alu.mult)
            e_stt(U2YW[:], U2YW[:], 0.5, RDS6[:], Alu.mult, Alu.mult)

            # w = u2 + p  (E = |w|^2) ; li/lj duals
            e_tt(w3v(W0, 6), w3v(U2XW, 6), bc(p1, 6), Alu.add)
            e_tt(w3v(W1, 6), w3v(U2YW, 6), bc(p2, 6), Alu.add)
            for pi, (i, j) in enumerate(PAIRS):
                t1 = tmp()
                e_tt(t1[:], slab(W0, pi), slab(G2W, j), Alu.mult)
                t2 = tmp()
                e_tt(t2[:], slab(W1, pi), slab(G1W, j), Alu.mult)
                e_tt(slab(LH, pi), t1[:], t2[:], Alu.subtract)
                t3 = tmp()
                e_tt(t3[:], slab(G1W, i), slab(W1, pi), Alu.mult)
                t4 = tmp()
                e_tt(t4[:], slab(G2W, i), slab(W0, pi), Alu.mult)
                e_tt(slab(LI2, pi), t3[:], t4[:], Alu.subtract)
            e_stt(LH[:], LH[:], 0.5, RDS6[:], Alu.mult, Alu.mult)
            e_stt(LI2[:], LI2[:], 0.5, RDS6[:], Alu.mult, Alu.mult)
            e_ts(LH[:], LH[:], -_f(TOL), None, Alu.is_ge)
            e_ts(LI2[:], LI2[:], -_f(TOL), None, Alu.is_ge)
            e_tt(LH[:], LH[:], LI2[:], Alu.mult)
            e_tt(LH[:], LH[:], OKF6[:], Alu.mult)          # dual2 & ok

            # E before feasibility (frees W0/W1 after)
            e_sq(EM6[:], W0[:])
            e_sq(LI2[:], W1[:])
            e_tt(EM6[:], EM6[:], LI2[:], Alu.add)          # E

            # feasibility at the two non-active constraints (wide)
            HTM1, HTM2 = W0, W1                            # reuse
            for pi in range(6):
                m1, m2 = MSETS[pi]
                e_copy(slab(HTM1, pi), slab(HTW, m1))
                e_copy(slab(HTM2, pi), slab(HTW, m2))
            MG1, MG2 = G1M1, G1M2                          # reuse in place
            e_tt(MG1[:], U2XW[:], G1M1[:], Alu.mult)
            e_tt(G2M1[:], U2YW[:], G2M1[:], Alu.mult)
            e_tt(MG1[:], MG1[:], G2M1[:], Alu.add)
            e_tt(MG1[:], MG1[:], HTM1[:], Alu.add)         # margin m1
            e_tt(MG2[:], U2XW[:], G1M2[:], Alu.mult)
            e_tt(G2M2[:], U2YW[:], G2M2[:], Alu.mult)
            e_tt(MG2[:], MG2[:], G2M2[:], Alu.add)
            e_tt(MG2[:], MG2[:], HTM2[:], Alu.add)         # margin m2
            e_tt(MG1[:], MG1[:], MG2[:], Alu.min, eng="v")
            e_ts(MG1[:], MG1[:], 0.0, None, Alu.is_ge)
            e_tt(LH[:], LH[:], MG1[:], Alu.mult)           # full mask

            e_ts(MG2[:], LH[:], -1e30, 1e30, Alu.mult, Alu.add)
            e_tt(EM6[:], EM6[:], MG2[:], Alu.add)          # E masked
            for pi in range(6):
                fold_candidate(slab(EM6, pi), slab(U2XW, pi), slab(U2YW, pi))

            # ---------------- output ----------------
            outsb = wp.tile([128, PC * 2], F32, tag="outsb", name="outsb")
            o3 = outsb[:].rearrange("p (c two) -> p c two", two=2)
            nc.vector.tensor_copy(o3[:, :, 0], bx[:])
            nc.vector.tensor_copy(o3[:, :, 1], by[:])
            nc.sync.dma_start(
                out=out_d.ap().rearrange("(p c) two -> p (c two)", p=128),
                in_=outsb[:])
    nc.compile()
    return nc


def _host_prep(inputs):
    """Build lhsT weight layouts and consts."""
    mean = np.asarray(inputs["mean"], np.float32)
    W1 = np.asarray(inputs["W1"], np.float32)
    b1 = np.asarray(inputs["b1"], np.float32)
    W21 = np.asarray(inputs["W21"], np.float32)
    b21 = np.asarray(inputs["b21"], np.float32)
    W22 = np.asarray(inputs["W22"], np.float32)
    b22 = np.asarray(inputs["b22"], np.float32)
    W31 = np.asarray(inputs["W31"], np.float32)
    b31 = np.asarray(inputs["b31"], np.float32)
    W32 = np.asarray(inputs["W32"], np.float32)
    b32 = np.asarray(inputs["b32"], np.float32)

    w1t = np.ascontiguousarray(W1.T).astype(np.float32)            # [8, 256]

    Wcat = np.vstack([W21, W22]).astype(np.float32)                # [256, 256]
    wcat = np.concatenate([Wcat[:, :128].T, Wcat[:, 128:].T],
                          axis=1)                                  # [128, 512]
    wcat = np.ascontiguousarray(wcat, dtype=np.float32)
    bcat = np.concatenate([b21, b22]).astype(np.float32)

    W3blk = np.zeros((4, 256), np.float32)
    W3blk[0:2, 0:128] = W31
    W3blk[2:4, 128:256] = W32
    w3t = np.concatenate([W3blk[:, :128].T, W3blk[:, 128:].T],
                         axis=1)                                   # [128, 8]
    w3t = np.ascontiguousarray(w3t, dtype=np.float32)
    b3 = np.concatenate([b31, b32]).astype(np.float32)

    consts = np.zeros((128, 16), np.float32)
    consts[:, 0] = b1[:128]
    consts[:, 1] = b1[128:]
    consts[:, 2] = bcat[:128]
    consts[:, 3] = bcat[128:]
    consts[:, 4] = b3[3]      # b32[1] (sigmoid bias for s2)
    consts[:, 13] = b3[0]     # b31[0]
    consts[:, 14] = b3[1]     # b31[1]
    consts[:, 15] = b3[2]     # b32[0] (sigmoid bias for s1)
    return w1t, wcat, w3t, consts


def kernel(**inputs):
    x = np.ascontiguousarray(np.asarray(inputs["x"], np.float32))
    assert x.shape == (B, NF)
    w1t, wcat, w3t, consts = _host_prep(inputs)

    zb = (not np.any(np.asarray(inputs["b1"]))
          and not np.any(np.asarray(inputs["b21"]))
          and not np.any(np.asarray(inputs["b22"])))
    key = ("nc", zb)
    if key not in _NC_CACHE:
        _NC_CACHE[key] = _build_nc(zero_bias=zb)
    nc = _NC_CACHE[key]

    in_maps = []
    for c in range(NCORES):
        xs = x[c * SHARD:(c + 1) * SHARD]
        in_maps.append({
            "x": xs,
            "xt": np.ascontiguousarray(xs.T),
            "w1t": w1t, "wcat": wcat, "w3t": w3t,
            "consts": consts,
        })
    res = run_bass_kernel_spmd(nc, in_maps, list(range(NCORES)))
    out = np.concatenate([res.results[c]["out"] for c in range(NCORES)], axis=0)
    return out.astype(np.float32)


# revision 30
# speedup vs baseline: 1.0644x; 1.0644x over previous
"""BarrierNet Trainium2 kernel: MLP + batched 2-var QP (active-set enumeration).

Self-contained: shards B=262144 samples across 8 NeuronCores (data parallel),
runs a Bass/Tile kernel per core, gathers the full output.

v2 layout per core (SHARD = 32768 samples):
  - MLP: 64 tiles of 512 samples, feature-major fp32r matmuls, software
    pipelined so the PE never waits (L1(t+1) | L2(t) | L3(t-1) interleave).
    xT comes pre-transposed from the host and is DMA'd straight into f32r.
  - All x-only QP geometry (G, q, S, barrier, det, reciprocal dens, wide
    operand arrangements) is "dripped" between MLP tiles so DVE/ACT/Pool
    fill their idle time under the PE-bound phase.
  - The z-dependent QP tail (p, sigmoids -> hq -> 11-candidate enumeration)
    compares candidates on E = |u - u0|^2 (obj(u) = -0.5|p|^2 + 0.5 E) and
    is engine-balanced with a measured-cost greedy.
"""
import numpy as np

import concourse.bass as bass
import concourse.bacc as bacc
import concourse.tile as tile
from concourse import mybir
from concourse.bass_utils import run_bass_kernel_spmd

F32 = mybir.dt.float32
F32R = mybir.dt.float32r
U8 = mybir.dt.uint8
Alu = mybir.AluOpType
Act = mybir.ActivationFunctionType

NCORES = 8
B = 262144
SHARD = B // NCORES            # 32768
PC = SHARD // 128              # 256  (plane free dim)
TILE_N = 512
NT = SHARD // TILE_N           # 64
NF, H1, H2 = 8, 256, 128

STATIC_OBS = np.array([[8.0, -8.0, 1.0], [-9.0, 7.0, 1.0], [10.0, 10.0, 1.5]],
                      np.float32)
AGENT_R, SAFETY = np.float32(0.5), np.float32(0.1)
TOL = 1e-6
PAIRS = [(0, 1), (0, 2), (0, 3), (1, 2), (1, 3), (2, 3)]

_NC_CACHE = {}


def _f(x):
    return float(np.float32(x))


def _build_nc(zero_bias=False):
    nc = bacc.Bacc("TRN2", target_bir_lowering=False, debug=False,
                   num_devices=NCORES)

    x_d = nc.dram_tensor("x", [SHARD, NF], F32, kind="ExternalInput")
    xt_d = nc.dram_tensor("xt", [NF, SHARD], F32R, kind="ExternalInput")
    w1t_d = nc.dram_tensor("w1t", [NF, H1], F32R, kind="ExternalInput")
    wcat_d = nc.dram_tensor("wcat", [128, 512], F32R, kind="ExternalInput")
    w3t_d = nc.dram_tensor("w3t", [128, 8], F32R, kind="ExternalInput")
    consts_d = nc.dram_tensor("consts", [128, 16], F32, kind="ExternalInput")
    out_d = nc.dram_tensor("out", [SHARD, 2], F32, kind="ExternalOutput")

    W4, W6 = 4 * PC, 6 * PC

    with tile.TileContext(nc) as tc:
        with tc.tile_pool(name="wpool", bufs=1) as wp, \
             tc.tile_pool(name="mlp", bufs=2) as mp, \
             tc.tile_pool(name="planes", bufs=1) as pp, \
             tc.tile_pool(name="tmps", bufs=2) as tp, \
             tc.tile_pool(name="psA", bufs=1, space="PSUM") as psA, \
             tc.tile_pool(name="psB", bufs=2, space="PSUM") as psB, \
             tc.tile_pool(name="psC", bufs=2, space="PSUM") as psC, \
             tc.tile_pool(name="dram", bufs=1, space="DRAM") as dp:

            # ---------------- engine-balance helpers ----------------
            eng_load = {"v": 0.0, "a": 0.0, "g": 0.0}
            POOL_TT_OK = (Alu.add, Alu.subtract, Alu.mult)

            def _fd(ap):
                return ap.free_size()

            def _pick(costs):
                e = min(costs, key=lambda k: eng_load[k] + costs[k])
                eng_load[e] += costs[e]
                return e

            def _take(e, cost):
                eng_load[e] += cost

            def e_tt(out, a, b, op, eng=None):
                """tensor-tensor; min/cmp only on DVE; bcast only on DVE."""
                fd = _fd(out)
                if eng is None:
                    if op in POOL_TT_OK:
                        eng = _pick({"v": 60 + fd * 1.042,
                                     "g": 100 + fd * 2.083})
                    else:
                        eng = "v"
                        _take("v", 60 + fd * 1.042)
                else:
                    _take(eng, 60 + fd * 1.042 if eng == "v"
                          else 100 + fd * 2.083)
                (nc.vector if eng == "v" else nc.gpsimd).tensor_tensor(
                    out, a, b, op)
                return out

            def e_ts(out, a, s1, s2, op0, op1=None, eng=None):
                fd = _fd(out)
                s1 = s1 if isinstance(s1, bass.AP) else _f(s1)
                if s2 is not None:
                    s2 = s2 if isinstance(s2, bass.AP) else _f(s2)
                if eng is None:
                    eng = _pick({"v": 60 + fd * 0.521, "g": 100 + fd * 1.48})
                else:
                    _take(eng, 60 + fd * 0.521 if eng == "v"
                          else 100 + fd * 1.48)
                e = nc.vector if eng == "v" else nc.gpsimd
                if s2 is None:
                    e.tensor_scalar(out, a, s1, None, op0)
                else:
                    e.tensor_scalar(out, a, s1, s2, op0, op1)
                return out

            def e_stt(out, a, s, b, op0, op1):
                _take("v", 60 + _fd(out) * 1.042)
                nc.vector.scalar_tensor_tensor(out, a, _f(s), b, op0, op1)
                return out

            def e_un(out, a, func, bias=0.0, scale=1.0):
                _take("a", 255 + _fd(out) * 0.833)
                nc.scalar.activation(out, a, func, bias=bias, scale=scale)
                return out

            def e_sq(out, a, scale=None):
                fd = _fd(out)
                if scale is None:
                    eng = _pick({"v": 60 + fd * 1.042,
                                 "g": 100 + fd * 2.083,
                                 "a": 255 + fd * 0.833})
                else:
                    eng = "a"
                    _take("a", 255 + fd * 0.833)
                if eng == "a":
                    nc.scalar.activation(out, a, Act.Square,
                                         scale=1.0 if scale is None else scale)
                else:
                    (nc.vector if eng == "v" else nc.gpsimd).tensor_tensor(
                        out, a, a, Alu.mult)
                return out

            def e_copy(out, a, eng=None):
                fd = _fd(out)
                if eng is None:
                    eng = _pick({"v": 60 + fd * 0.521,
                                 "g": 100 + fd * 1.48,
                                 "a": 255 + fd * 0.833})
                else:
                    _take(eng, 100 + fd * 1.48)
                if eng == "a":
                    nc.scalar.activation(out, a, Act.Copy)
                else:
                    (nc.vector if eng == "v" else nc.gpsimd).tensor_copy(out, a)
                return out

            def e_recip(out, a):
                _take("v", 60 + _fd(out) * 1.042)
                nc.vector.reciprocal_approx_fast(out, a)
                return out

            # ---------------- tiles: constants / weights ----------------
            cs = wp.tile([128, 16], F32, tag="consts", name="consts")
            nc.sync.dma_start(out=cs[:], in_=consts_d.ap())
            w1t_r = wp.tile([NF, H1], F32R, tag="w1tr", name="w1tr")
            wcat_r = wp.tile([128, 512], F32R, tag="wcatr", name="wcatr")
            w3t_r = wp.tile([128, 8], F32R, tag="w3tr", name="w3tr")
            nc.sync.dma_start(out=w1t_r[:], in_=w1t_d.ap())
            nc.sync.dma_start(out=wcat_r[:], in_=wcat_d.ap())
            nc.sync.dma_start(out=w3t_r[:], in_=w3t_d.ap())

            # x planes (f-interleaved) for geometry
            xbig = wp.tile([128, PC * NF], F32, tag="xbig", name="xbig")
            nc.sync.dma_start(
                out=xbig[:], in_=x_d.ap().rearrange("(p c) f -> p (c f)", p=128))
            xb3 = xbig[:].rearrange("p (c f) -> p c f", f=NF)

            def xf(i):
                return xb3[:, :, i]

            z3_dram = dp.tile([4, SHARD], F32, tag="z3d", name="z3d")

            # ---------------- persistent QP tiles ----------------
            def w4t(tag):
                return pp.tile([128, W4], F32, tag=tag, name=tag)

            def w6t(tag):
                return pp.tile([128, W6], F32, tag=tag, name=tag)

            def plane(tag):
                return pp.tile([128, PC], F32, tag=tag, name=tag)

            def tmp():
                tmp._i += 1
                t = f"tmp{tmp._i % 6}"
                return tp.tile([128, PC], F32, tag=t, name=t, bufs=2)

            tmp._i = 0

            def slab(w, i):
                return w[:, i * PC:(i + 1) * PC]

            def bc(plane_ap, n):
                return plane_ap.rearrange(
                    "p (o c) -> p o c", o=1).to_broadcast((128, n, PC))

            def w3v(w, n=4):
                return w[:].rearrange("p (o c) -> p o c", o=n)

            # x-only wide4: A (bd), BAR16, G1W, G2W, QW, RDENW
            AW, BARW = w4t("AW"), w4t("BARW")
            G1W, G2W = w4t("G1W"), w4t("G2W")
            QW, RDENW = w4t("QW"), w4t("RDENW")
            # x-only wide6
            RDS6, OKF6 = w6t("RDS6"), w6t("OKF6")
            G1M1, G2M1 = w6t("G1M1"), w6t("G2M1")
            G1M2, G2M2 = w6t("G1M2"), w6t("G2M2")
            OKI6 = pp.tile([128, W6], U8, tag="OKI6", name="OKI6")

            # ---------------- x-only geometry (deferred drip) ----------
            XOPS = []

            def xop(f):
                XOPS.append(f)

            mu = [0.0, 0.0, 0.0, 1.0, 6.0, 6.0]
            sg = [1.0, 1.0, 0.5, 0.3, 1.0, 1.0]
            rtot = np.concatenate(
                [AGENT_R + STATIC_OBS[:, 2] + SAFETY,
                 np.array([2 * AGENT_R + SAFETY], np.float32)]).astype(np.float32)
            r2 = (rtot * rtot).astype(np.float32)

            DXW, DYW = w4t("DXW"), w4t("DYW")
            st, ct, v = plane("st"), plane("ct"), plane("v")
            vst, vct, h0 = plane("vst"), plane("vct"), plane("h0")
            oxo, oyo, sh, sh2 = tmp(), tmp(), tmp(), tmp()

            for k in range(3):
                xop(lambda k=k: e_ts(slab(DXW, k), xf(0), sg[0],
                                     mu[0] - STATIC_OBS[k, 0], Alu.mult,
                                     Alu.add, eng="g"))
                xop(lambda k=k: e_ts(slab(DYW, k), xf(1), sg[1],
                                     mu[1] - STATIC_OBS[k, 1], Alu.mult,
                                     Alu.add, eng="g"))
            xop(lambda: e_ts(oxo[:], xf(4), sg[4], mu[4] - mu[0], Alu.mult,
                             Alu.add, eng="g"))
            xop(lambda: e_ts(oyo[:], xf(5), sg[5], mu[5] - mu[1], Alu.mult,
                             Alu.add, eng="g"))
            # dx_opp = x0*sg0 - oxo  (no Pool STT: two steps via slab reuse)
            xop(lambda: e_ts(slab(DXW, 3), xf(0), sg[0], None, Alu.mult,
                             eng="g"))
            xop(lambda: e_tt(slab(DXW, 3), slab(DXW, 3), oxo[:], Alu.subtract,
                             eng="g"))
            xop(lambda: e_ts(slab(DYW, 3), xf(1), sg[1], None, Alu.mult,
                             eng="g"))
            xop(lambda: e_tt(slab(DYW, 3), slab(DYW, 3), oyo[:], Alu.subtract,
                             eng="g"))

            # sins go on ACT before the MLP loop starts (ACT is free then);
            # everything dripped during MLP must be Pool-only so DVE/ACT
            # never queue behind a backlogged Pool producer.
            e_un(st[:], xf(2), Act.Sin, bias=0.0, scale=sg[2])
            e_un(sh[:], xf(2), Act.Sin, bias=0.0, scale=sg[2] * 0.5)
            xop(lambda: e_tt(sh2[:], sh[:], sh[:], Alu.mult, eng="g"))
            xop(lambda: e_ts(ct[:], sh2[:], -2.0, 1.0, Alu.mult, Alu.add,
                             eng="g"))
            xop(lambda: e_ts(v[:], xf(3), sg[3], float(mu[3]), Alu.mult,
                             Alu.add, eng="g"))
            xop(lambda: e_tt(vst[:], v[:], st[:], Alu.mult, eng="g"))
            xop(lambda: e_tt(vct[:], v[:], ct[:], Alu.mult, eng="g"))
            # h0 = 2 v^2
            xop(lambda: e_tt(h0[:], v[:], v[:], Alu.mult, eng="g"))
            xop(lambda: e_ts(h0[:], h0[:], 2.0, None, Alu.mult, eng="g"))

            # per-slab products: A = dx*vct + dy*vst ; G1 = dy*vct - dx*vst
            #                    G2 = dx*ct + dy*st
            for k in range(4):
                xop(lambda k=k: e_tt(slab(AW, k), slab(DXW, k), vct[:],
                                     Alu.mult, eng="g"))
                xop(lambda k=k: e_tt(slab(G1W, k), slab(DYW, k), vct[:],
                                     Alu.mult, eng="g"))
                xop(lambda k=k: e_tt(slab(G2W, k), slab(DXW, k), ct[:],
                                     Alu.mult, eng="g"))
            T1, T2 = tmp(), tmp()
            for k in range(4):
                xop(lambda k=k: e_tt(T1[:], slab(DYW, k), vst[:], Alu.mult,
                                     eng="g"))
                xop(lambda k=k: e_tt(slab(AW, k), slab(AW, k), T1[:], Alu.add,
                                     eng="g"))
                xop(lambda k=k: e_tt(T2[:], slab(DXW, k), vst[:], Alu.mult,
                                     eng="g"))
                xop(lambda k=k: e_tt(slab(G1W, k), slab(G1W, k), T2[:],
                                     Alu.subtract, eng="g"))
                xop(lambda k=k: e_tt(T1[:], slab(DYW, k), st[:], Alu.mult,
                                     eng="g"))
                xop(lambda k=k: e_tt(slab(G2W, k), slab(G2W, k), T1[:],
                                     Alu.add, eng="g"))
            # BAR16 = 16*(dx^2 + dy^2 - r2)
            for k in range(4):
                xop(lambda k=k: e_tt(slab(BARW, k), slab(DXW, k), slab(DXW, k),
                                     Alu.mult, eng="g"))
                xop(lambda k=k: e_tt(T2[:], slab(DYW, k), slab(DYW, k),
                                     Alu.mult, eng="g"))
                xop(lambda k=k: e_tt(slab(BARW, k), slab(BARW, k), T2[:],
                                     Alu.add, eng="g"))
                xop(lambda k=k: e_ts(slab(BARW, k), slab(BARW, k), 16.0,
                                     _f(-16.0 * r2[k]), Alu.mult, Alu.add,
                                     eng="g"))
            # QW = g1^2 + g2^2 ; RDEN = 1/(2q + eps)
            for k in range(4):
                xop(lambda k=k: e_tt(slab(QW, k), slab(G1W, k), slab(G1W, k),
                                     Alu.mult, eng="g"))
                xop(lambda k=k: e_tt(T2[:], slab(G2W, k), slab(G2W, k),
                                     Alu.mult, eng="g"))
                xop(lambda k=k: e_tt(slab(QW, k), slab(QW, k), T2[:], Alu.add,
                                     eng="g"))
            xop(lambda: e_ts(RDENW[:], QW[:], 2.0, 5e-10, Alu.mult, Alu.add,
                             eng="g"))
            XTAIL = []
            XTAIL.append(lambda: e_recip(RDENW[:], RDENW[:]))

            # S planes (j<k pairs)
            Spl = {}
            for (i, j) in PAIRS:
                s_ij = plane(f"S{i}{j}")
                Spl[(i, j)] = Spl[(j, i)] = s_ij
                xop(lambda i=i, j=j, s=s_ij: e_tt(s[:], slab(G1W, i),
                                                  slab(G1W, j), Alu.mult,
                                                  eng="g"))
                xop(lambda i=i, j=j, s=s_ij: e_tt(T1[:], slab(G2W, i),
                                                  slab(G2W, j), Alu.mult,
                                                  eng="g"))
                xop(lambda s=s_ij: e_tt(s[:], s[:], T1[:], Alu.add, eng="g"))

            # u2 det stage: det/4 = g1i*g2j - g2i*g1j ; RDS6 = 1/det_safe
            DET6 = w6t("DET6")
            for pi, (i, j) in enumerate(PAIRS):
                xop(lambda pi=pi, i=i, j=j: e_tt(slab(DET6, pi), slab(G1W, i),
                                                 slab(G2W, j), Alu.mult,
                                                 eng="g"))
                xop(lambda pi=pi, i=i, j=j: e_tt(T2[:], slab(G2W, i),
                                                 slab(G1W, j), Alu.mult,
                                                 eng="g"))
                xop(lambda pi=pi: e_tt(slab(DET6, pi), slab(DET6, pi), T2[:],
                                       Alu.subtract, eng="g"))
            XTAIL.append(lambda: e_un(OKF6[:], DET6[:], Act.Abs))
            XTAIL.append(lambda: e_ts(OKI6[:], OKF6[:], 2.5e-10, None,
                                      Alu.is_gt, eng="v"))
            XTAIL.append(lambda: e_ts(OKF6[:], OKF6[:], 2.5e-10, None,
                                      Alu.is_gt, eng="v"))
            XTAIL.append(lambda: e_ts(RDS6[:], OKF6[:], 0.0, 0.25, Alu.mult,
                                      Alu.add, eng="v"))
            XTAIL.append(lambda: (_take("v", 60 + W6 * 1.042),
                                  nc.vector.copy_predicated(RDS6[:], OKI6[:],
                                                            DET6[:]))[1])
            XTAIL.append(lambda: e_recip(RDS6[:], RDS6[:]))

            # feas-u2 G arrangements: for pair (i,j), m1/m2 = constraints
            # not in the pair
            MSETS = [[m for m in range(4) if m not in pr] for pr in PAIRS]
            for pi in range(6):
                m1, m2 = MSETS[pi]
                xop(lambda pi=pi, m=m1: e_copy(slab(G1M1, pi), slab(G1W, m),
                                               eng="g"))
                xop(lambda pi=pi, m=m1: e_copy(slab(G2M1, pi), slab(G2W, m),
                                               eng="g"))
                xop(lambda pi=pi, m=m2: e_copy(slab(G1M2, pi), slab(G1W, m),
                                               eng="g"))
                xop(lambda pi=pi, m=m2: e_copy(slab(G2M2, pi), slab(G2W, m),
                                               eng="g"))

            # ---------------- MLP pipelined loop ----------------
            ACT_SPLIT = 544          # relu cols on ACT; rest on DVE
            xt_tiles = [None] * NT
            h1ps = [None] * NT
            h2ps = [None] * NT
            h1r = [None] * NT
            h2r = [None] * NT
            z3ps = [None] * (NT // 2)

            emitted = [0]

            def drip(i):
                quota = (len(XOPS) * (i + 1)) // (NT + 2)
                while emitted[0] < quota:
                    XOPS[emitted[0]]()
                    emitted[0] += 1

            for i in range(NT + 2):
                if i < NT:
                    xt = mp.tile([NF, TILE_N], F32R, tag="xTr", name="xTr",
                                 bufs=2)
                    nc.sync.dma_start(
                        out=xt[:],
                        in_=xt_d.ap()[:, i * TILE_N:(i + 1) * TILE_N])
                    xt_tiles[i] = xt
                    h1_ps = psA.tile([128, 2 * TILE_N], F32, tag="h1ps",
                                     name="h1ps")
                    for mc in range(2):
                        nc.tensor.matmul(
                            h1_ps[:, mc * TILE_N:(mc + 1) * TILE_N],
                            w1t_r[:, mc * 128:(mc + 1) * 128],
                            xt[:], start=True, stop=True)
                    h1ps[i] = h1_ps
                if 1 <= i:
                    t = i - 1
                    if t < NT:
                        # h1 epilogue split ACT/DVE
                        h1r_t = mp.tile([128, 2 * TILE_N], F32R, tag="h1r",
                                        name="h1r")
                        hp = h1ps[t]
                        if zero_bias:
                            nc.scalar.activation(h1r_t[:, :ACT_SPLIT],
                                                 hp[:, :ACT_SPLIT], Act.Relu)
                            nc.vector.tensor_scalar(h1r_t[:, ACT_SPLIT:],
                                                    hp[:, ACT_SPLIT:],
                                                    0.0, None, Alu.max)
                        else:
                            nc.scalar.activation(h1r_t[:, 0:TILE_N],
                                                 hp[:, 0:TILE_N], Act.Relu,
                                                 bias=cs[:, 0:1], scale=1.0)
                            nc.vector.tensor_scalar(h1r_t[:, TILE_N:],
                                                    hp[:, TILE_N:],
                                                    cs[:, 1:2], 0.0,
                                                    Alu.add, Alu.max)
                        h1r[t] = h1r_t
                        h1ps[t] = None
                        # L2
                        h2_ps = psB.tile([128, 2 * TILE_N], F32, tag="h2ps",
                                         name="h2ps")
                        for mo in range(2):
                            for kc in range(2):
                                nc.tensor.matmul(
                                    h2_ps[:, mo * TILE_N:(mo + 1) * TILE_N],
                                    wcat_r[:, kc * 256 + mo * 128:
                                           kc * 256 + (mo + 1) * 128],
                                    h1r_t[:, kc * TILE_N:(kc + 1) * TILE_N],
                                    start=(kc == 0), stop=(kc == 1))
                        h2ps[t] = h2_ps
                if 2 <= i:
                    t = i - 2
                    if t < NT:
                        # h2 epilogue split ACT/DVE
                        h2r_t = mp.tile([128, 2 * TILE_N], F32R, tag="h2r",
                                        name="h2r")
                        hp = h2ps[t]
                        if zero_bias:
                            nc.scalar.activation(h2r_t[:, :ACT_SPLIT],
                                                 hp[:, :ACT_SPLIT], Act.Relu)
                            nc.vector.tensor_scalar(h2r_t[:, ACT_SPLIT:],
                                                    hp[:, ACT_SPLIT:],
                                                    0.0, None, Alu.max)
                        else:
                            nc.scalar.activation(h2r_t[:, 0:TILE_N],
                                                 hp[:, 0:TILE_N], Act.Relu,
                                                 bias=cs[:, 2:3], scale=1.0)
                            nc.vector.tensor_scalar(h2r_t[:, TILE_N:],
                                                    hp[:, TILE_N:],
                                                    cs[:, 3:4], 0.0,
                                                    Alu.add, Alu.max)
                        h2r[t] = h2r_t
                        h2ps[t] = None
                        h1r[t] = None
                        # L3
                        zp = psC.tile([4, TILE_N], F32, tag="z3ps",
                                      name="z3ps")
                        for kc in range(2):
                            nc.tensor.matmul(
                                zp[:],
                                w3t_r[:, kc * 4:(kc + 1) * 4],
                                h2r_t[:, kc * TILE_N:(kc + 1) * TILE_N],
                                start=(kc == 0), stop=(kc == 1))
                        z3_sb = mp.tile([4, TILE_N], F32, tag="z3sb",
                                        name="z3sb")
                        if t % 2 == 0:
                            nc.vector.tensor_copy(z3_sb[:], zp[:])
                        else:
                            nc.scalar.activation(z3_sb[:], zp[:], Act.Copy)
                        nc.sync.dma_start(
                            out=z3_dram[:, t * TILE_N:(t + 1) * TILE_N],
                            in_=z3_sb[:])
                drip(i)

            # ============ z-dependent QP tail ============
            # reset balancer (all engines free now)
            for k in eng_load:
                eng_load[k] = 0.0

            for f in XTAIL:
                f()

            zpl = []
            for iz in range(4):
                z = tmp()
                nc.sync.dma_start(
                    out=z[:],
                    in_=z3_dram[iz, :].rearrange("(p c) -> p c", p=128))
                zpl.append(z)
            zr1, zr2, zs1, zs2 = (z[:] for z in zpl)

            p1t, p2t = plane("p1"), plane("p2")
            e_un(p1t[:], zr1, Act.Identity, bias=cs[:, 13:14])
            e_un(p2t[:], zr2, Act.Identity, bias=cs[:, 14:15])
            p1, p2 = p1t[:], p2t[:]
            sg1, sg2p = plane("sg1"), plane("sg2")
            e_un(sg1[:], zs1, Act.Sigmoid, bias=cs[:, 15:16])
            e_un(sg2p[:], zs2, Act.Sigmoid, bias=cs[:, 4:5])
            sab8, smm = plane("sab8"), plane("smm")
            e_tt(sab8[:], sg1[:], sg2p[:], Alu.add)
            e_ts(sab8[:], sab8[:], 8.0, None, Alu.mult)
            e_tt(smm[:], sg1[:], sg2p[:], Alu.mult)

            # wide z-dep buffers
            HQW, HTW = w4t("HQW"), w4t("HTW")
            M0W, LAMW = w4t("M0W"), w4t("LAMW")
            U1XW, U1YW = DXW, DYW          # reuse (geometry dead)
            C4, D4 = w4t("C4"), w4t("D4")

            # HQ = h0 + sab8*A + smm*BAR16
            e_tt(w3v(HQW), bc(sab8[:], 4), w3v(AW), Alu.mult)
            e_tt(w3v(C4), bc(smm[:], 4), w3v(BARW), Alu.mult)
            e_tt(HQW[:], HQW[:], C4[:], Alu.add)
            e_tt(w3v(HQW), w3v(HQW), bc(h0[:], 4), Alu.add)
            e_ts(HTW[:], HQW[:], 0.5, _f(TOL * 0.5), Alu.mult, Alu.add)

            # t_k = g1*p1 + g2*p2 ; M0 = HT - t ; nu = 2t - hq ; lam = nu*rden
            e_tt(w3v(C4), w3v(G1W), bc(p1, 4), Alu.mult)
            e_tt(w3v(D4), w3v(G2W), bc(p2, 4), Alu.mult)
            e_tt(C4[:], C4[:], D4[:], Alu.add)                 # t_k
            e_tt(M0W[:], HTW[:], C4[:], Alu.subtract)
            e_stt(C4[:], C4[:], 2.0, HQW[:], Alu.mult, Alu.subtract)  # nu
            e_tt(LAMW[:], C4[:], RDENW[:], Alu.mult)
            MKW = RDENW                    # reuse (rden dead after lam)

            # candidate 0
            fmin0, flag0 = tmp(), tmp()
            nc.vector.tensor_reduce(
                fmin0[:], M0W[:].rearrange("p (k c) -> p c k", k=4),
                mybir.AxisListType.X, Alu.min)
            _take("v", 60 + W4 * 1.042)
            bo, bx, by = plane("best_obj"), plane("best_ux"), plane("best_uy")
            e_ts(flag0[:], fmin0[:], 0.0, None, Alu.is_ge)
            e_ts(bo[:], flag0[:], -1e30, 1e30, Alu.mult, Alu.add)
            e_ts(bx[:], p1, -1.0, None, Alu.mult)
            e_ts(by[:], p2, -1.0, None, Alu.mult)

            def fold_candidate(objm, ux, uy):
                bt = tp.tile([128, PC], U8, tag="bt", name="bt", bufs=2)
                e_tt(bt[:], objm, bo[:], Alu.is_lt, eng="v")
                _take("v", 2 * (60 + PC * 1.042))
                nc.vector.copy_predicated(bx[:], bt[:], ux)
                nc.vector.copy_predicated(by[:], bt[:], uy)
                e_tt(bo[:], objm, bo[:], Alu.min, eng="v")

            # u1 candidates
            e_tt(U1XW[:], LAMW[:], G1W[:], Alu.mult)
            e_tt(w3v(U1XW), w3v(U1XW), bc(p1, 4), Alu.subtract)
            e_tt(U1YW[:], LAMW[:], G2W[:], Alu.mult)
            e_tt(w3v(U1YW), w3v(U1YW), bc(p2, 4), Alu.subtract)
            e_ts(MKW[:], LAMW[:], -_f(2 * TOL), None, Alu.is_ge)  # dual flag

            # u1 feasibility via S: margin_j = M0_j + lam_k * S_jk
            for k in range(4):
                fm = None
                for j in range(4):
                    if j == k:
                        continue
                    e1 = tmp()
                    e_tt(e1[:], slab(LAMW, k), Spl[(j, k)][:], Alu.mult)
                    mg = tmp()
                    e_tt(mg[:], e1[:], slab(M0W, j), Alu.add)
                    if fm is None:
                        fm = mg
                    else:
                        e_tt(fm[:], fm[:], mg[:], Alu.min, eng="v")
                ff = tmp()
                e_ts(ff[:], fm[:], 0.0, None, Alu.is_ge)
                e_tt(slab(MKW, k), slab(MKW, k), ff[:], Alu.mult)

            # E = lam^2 * q, masked
            e_sq(C4[:], LAMW[:])
            e_tt(C4[:], C4[:], QW[:], Alu.mult)
            e_ts(D4[:], MKW[:], -1e30, 1e30, Alu.mult, Alu.add)
            e_tt(C4[:], C4[:], D4[:], Alu.add)
            for k in range(4):
                fold_candidate(slab(C4, k), slab(U1XW, k), slab(U1YW, k))

            # ---------------- u2 candidates (6 pairs) ----------------
            U2XW, U2YW = w6t("U2XW"), w6t("U2YW")
            W0, W1 = w6t("W0"), w6t("W1")
            LH, LI2 = w6t("LH"), w6t("LI2")
            EM6 = DET6                                  # reuse (det dead)

            # e = hq_i g2_j - hq_j g2_i ; e2 = g1_j hq_i - g1_i hq_j
            for pi, (i, j) in enumerate(PAIRS):
                t1 = tmp()
                e_tt(t1[:], slab(HQW, i), slab(G2W, j), Alu.mult)
                t2 = tmp()
                e_tt(t2[:], slab(HQW, j), slab(G2W, i), Alu.mult)
                e_tt(slab(U2XW, pi), t1[:], t2[:], Alu.subtract)
                t3 = tmp()
                e_tt(t3[:], slab(G1W, j), slab(HQW, i), Alu.mult)
                t4 = tmp()
                e_tt(t4[:], slab(G1W, i), slab(HQW, j), Alu.mult)
                e_tt(slab(U2YW, pi), t3[:], t4[:], Alu.subtract)
            e_stt(U2XW[:], U2XW[:], -0.5, RDS6[:], Alu.mult, Alu.mult)
            e_stt(U2YW[:], U2YW[:], 0.5, RDS6[:], Alu.mult, Alu.mult)

            # w = u2 + p  (E = |w|^2) ; li/lj duals
            e_tt(w3v(W0, 6), w3v(U2XW, 6), bc(p1, 6), Alu.add)
            e_tt(w3v(W1, 6), w3v(U2YW, 6), bc(p2, 6), Alu.add)
            for pi, (i, j) in enumerate(PAIRS):
                t1 = tmp()
                e_tt(t1[:], slab(W0, pi), slab(G2W, j), Alu.mult)
                t2 = tmp()
                e_tt(t2[:], slab(W1, pi), slab(G1W, j), Alu.mult)
                e_tt(slab(LH, pi), t1[:], t2[:], Alu.subtract)
                t3 = tmp()
                e_tt(t3[:], slab(G1W, i), slab(W1, pi), Alu.mult)
                t4 = tmp()
                e_tt(t4[:], slab(G2W, i), slab(W0, pi), Alu.mult)
                e_tt(slab(LI2, pi), t3[:], t4[:], Alu.subtract)
            e_stt(LH[:], LH[:], 0.5, RDS6[:], Alu.mult, Alu.mult)
            e_stt(LI2[:], LI2[:], 0.5, RDS6[:], Alu.mult, Alu.mult)
            e_ts(LH[:], LH[:], -_f(TOL), None, Alu.is_ge)
            e_ts(LI2[:], LI2[:], -_f(TOL), None, Alu.is_ge)
            e_tt(LH[:], LH[:], LI2[:], Alu.mult)
            e_tt(LH[:], LH[:], OKF6[:], Alu.mult)          # dual2 & ok

            # E before feasibility (frees W0/W1 after)
            e_sq(EM6[:], W0[:])
            e_sq(LI2[:], W1[:])
            e_tt(EM6[:], EM6[:], LI2[:], Alu.add)          # E

            # feasibility at the two non-active constraints (wide)
            HTM1, HTM2 = W0, W1                            # reuse
            for pi in range(6):
                m1, m2 = MSETS[pi]
                e_copy(slab(HTM1, pi), slab(HTW, m1))
                e_copy(slab(HTM2, pi), slab(HTW, m2))
            MG1, MG2 = G1M1, G1M2                          # reuse in place
            e_tt(MG1[:], U2XW[:], G1M1[:], Alu.mult)
            e_tt(G2M1[:], U2YW[:], G2M1[:], Alu.mult)
            e_tt(MG1[:], MG1[:], G2M1[:], Alu.add)
            e_tt(MG1[:], MG1[:], HTM1[:], Alu.add)         # margin m1
            e_tt(MG2[:], U2XW[:], G1M2[:], Alu.mult)
            e_tt(G2M2[:], U2YW[:], G2M2[:], Alu.mult)
            e_tt(MG2[:], MG2[:], G2M2[:], Alu.add)
            e_tt(MG2[:], MG2[:], HTM2[:], Alu.add)         # margin m2
            e_tt(MG1[:], MG1[:], MG2[:], Alu.min, eng="v")
            e_ts(MG1[:], MG1[:], 0.0, None, Alu.is_ge)
            e_tt(LH[:], LH[:], MG1[:], Alu.mult)           # full mask

            e_ts(MG2[:], LH[:], -1e30, 1e30, Alu.mult, Alu.add)
            e_tt(EM6[:], EM6[:], MG2[:], Alu.add)          # E masked
            for pi in range(6):
                fold_candidate(slab(EM6, pi), slab(U2XW, pi), slab(U2YW, pi))

            # ---------------- output ----------------
            outsb = wp.tile([128, PC * 2], F32, tag="outsb", name="outsb")
            o3 = outsb[:].rearrange("p (c two) -> p c two", two=2)
            nc.vector.tensor_copy(o3[:, :, 0], bx[:])
            nc.vector.tensor_copy(o3[:, :, 1], by[:])
            nc.sync.dma_start(
                out=out_d.ap().rearrange("(p c) two -> p (c two)", p=128),
                in_=outsb[:])
    nc.compile()
    return nc


def _host_prep(inputs):
    """Build lhsT weight layouts and consts."""
    mean = np.asarray(inputs["mean"], np.float32)
    W1 = np.asarray(inputs["W1"], np.float32)
    b1 = np.asarray(inputs["b1"], np.float32)
    W21 = np.asarray(inputs["W21"], np.float32)
    b21 = np.asarray(inputs["b21"], np.float32)
    W22 = np.asarray(inputs["W22"], np.float32)
    b22 = np.asarray(inputs["b22"], np.float32)
    W31 = np.asarray(inputs["W31"], np.float32)
    b31 = np.asarray(inputs["b31"], np.float32)
    W32 = np.asarray(inputs["W32"], np.float32)
    b32 = np.asarray(inputs["b32"], np.float32)

    w1t = np.ascontiguousarray(W1.T).astype(np.float32)            # [8, 256]

    Wcat = np.vstack([W21, W22]).astype(np.float32)                # [256, 256]
    wcat = np.concatenate([Wcat[:, :128].T, Wcat[:, 128:].T],
                          axis=1)                                  # [128, 512]
    wcat = np.ascontiguousarray(wcat, dtype=np.float32)
    bcat = np.concatenate([b21, b22]).astype(np.float32)

    W3blk = np.zeros((4, 256), np.float32)
    W3blk[0:2, 0:128] = W31
    W3blk[2:4, 128:256] = W32
    w3t = np.concatenate([W3blk[:, :128].T, W3blk[:, 128:].T],
                         axis=1)                                   # [128, 8]
    w3t = np.ascontiguousarray(w3t, dtype=np.float32)
    b3 = np.concatenate([b31, b32]).astype(np.float32)

    consts = np.zeros((128, 16), np.float32)
    consts[:, 0] = b1[:128]
    consts[:, 1] = b1[128:]
    consts[:, 2] = bcat[:128]
    consts[:, 3] = bcat[128:]
    consts[:, 4] = b3[3]      # b32[1] (sigmoid bias for s2)
    consts[:, 13] = b3[0]     # b31[0]
    consts[:, 14] = b3[1]     # b31[1]
    consts[:, 15] = b3[2]     # b32[0] (sigmoid bias for s1)
    return w1t, wcat, w3t, consts


def kernel(**inputs):
    x = np.ascontiguousarray(np.asarray(inputs["x"], np.float32))
    assert x.shape == (B, NF)
    w1t, wcat, w3t, consts = _host_prep(inputs)

    zb = (not np.any(np.asarray(inputs["b1"]))
          and not np.any(np.asarray(inputs["b21"]))
          and not np.any(np.asarray(inputs["b22"])))
    key = ("nc", zb)
    if key not in _NC_CACHE:
        _NC_CACHE[key] = _build_nc(zero_bias=zb)
    nc = _NC_CACHE[key]

    in_maps = []
    for c in range(NCORES):
        xs = x[c * SHARD:(c + 1) * SHARD]
        in_maps.append({
            "x": xs,
            "xt": np.ascontiguousarray(xs.T),
            "w1t": w1t, "wcat": wcat, "w3t": w3t,
            "consts": consts,
        })
    res = run_bass_kernel_spmd(nc, in_maps, list(range(NCORES)))
    out = np.concatenate([res.results[c]["out"] for c in range(NCORES)], axis=0)
    return out.astype(np.float32)
